# revision 1
# baseline (speedup 1.0000x reference)
"""Causal single-head attention (B=16, T=2048, C=1024, H=64) on 8 TRN2 NeuronCores.

Strategy (vs v1 baseline, ~118.9us -> ~110us):
- Data-parallel over batch: 2 batches per core, weights replicated; the two
  batches are interleaved per 512-wide time slice so the PE always has
  independent work while exp/cast chains of the other batch drain.
- Projections: packed [Wq.T|Wk.T] stationary -> QK^T [128, T]; V^T via a
  column-tiled pair of accumulation chains (even C-chunks -> array cols 0:63,
  odd -> 64:127, concurrent in the PE) then one DVE merge-add. Both chain
  heads need start=True: PSUM has_written clears are region-scoped, so an
  uncovered first write would accumulate stale junk (nondeterministic bug).
- Attention transposed: S^T chunks land in 2-bank PSUM regions (A/B,
  alternating) so exp() runs as 20 large ACT instructions per batch instead
  of 40 small ones (ScalarE activation time 51us -> 37us/core).
- Diagonal chunks packed at causal widths into two groups (d0@0,d1@512 |
  d2@0,d3@256), masked with one DVE multiply each against a packed constant.
- PV accumulates into o_ps [80, 512] (64 V dims + denominator row + pad),
  double-buffered across slices so batch b1's PV does not wait on b0's
  finalize cast.
- bf16 everywhere on the PE (v1 used fp32 HIGH-mode final transposes);
  V-transposes ride the qk psum bank, finalize transposes ride the v bank.
- kt is built by a DVE copy, NOT a DMA: an SBUF->SBUF DMA queues behind the
  bulk x-load HBM traffic (~6us latency) and stalled the PE every slice.
- All DMA issue off ScalarE/VectorE (x loads: gpsimd, y stores per-slice:
  gpsimd); per-batch x loads issued ahead with pool back-pressure (bufs=4).
- PE warmup on an unwritten junk tile lifts the HAM clock gate (1.2->2.4GHz)
  before the first projection; a dummy exp preloads the ACT table set.
"""
import os
import sys

for _p in ("/opt/trn_rl_repo", "/root/.axon_site/_ro/trn_rl_repo"):
    if os.path.isdir(_p) and _p not in sys.path:
        sys.path.insert(0, _p)

import numpy as np
import ml_dtypes
import concourse.bacc as bacc
import concourse.mybir as mybir
from concourse.tile import TileContext
from concourse import bass_utils

F32 = mybir.dt.float32
BF16 = mybir.dt.bfloat16
EXP = mybir.ActivationFunctionType.Exp

B, T, C, H = 16, 2048, 1024, 64
NCORES = 8
BPC = B // NCORES          # batches per core
NTS = T // 512             # 4 t/q slices of 512
NCH = C // 128             # 8 contraction chunks
M_O = H + 1                # 65: V dims + denominator row
P_O = 80                   # padded height/stride (xbar needs multiple of 16)

USE_VCOLTILE = True

# Per-slice group plan: list of (region, chunks). region 'A' = [128,1536]
# (3 psum banks), 'B' = [128,1024] (2 banks). 'diag' expands to the packed
# causal layout below. Regions alternate so S-matmuls of group g+1 overlap
# the exp of group g.
PLAN = {
    0: [('A', 'diag1'), ('B', 'diag2')],
    1: [('A', [0, 1]), ('B', 'diag1'), ('A', 'diag2'), ('B', [2, 3])],
    2: [('A', [0, 1]), ('B', [2, 3]), ('A', 'diag1'), ('B', 'diag2'),
        ('A', [4, 5]), ('B', [6, 7])],
    3: [('A', [0, 1]), ('B', [2, 3]), ('A', [4, 5]), ('B', 'diag1'),
        ('A', 'diag2'), ('B', [6, 7]), ('A', [8, 9]), ('B', [10, 11])],
}
# diag chunks split into two <=2-bank groups: (d, col, width), plus the mask
# column range each occupies in the packed mask constant.
DIAG1_LAYOUT = [(0, 0, 512), (1, 512, 384)]    # mask cols 0:896
DIAG2_LAYOUT = [(2, 0, 256), (3, 256, 128)]    # mask cols 896:1280
DIAG_COLS = 1280

LAST_EXEC_TIME_NS = None
LAST_RESULTS = None


def _expand_groups(j):
    """Yield (region, used_cols, mask_range, [(chunk_i, col, width, qoff)])."""
    for region, chunks in PLAN[j]:
        if chunks == 'diag1':
            items = [(4 * j + d, col, w, 128 * d) for d, col, w in DIAG1_LAYOUT]
            yield region, 896, (0, 896), items
        elif chunks == 'diag2':
            items = [(4 * j + d, col, w, 128 * d) for d, col, w in DIAG2_LAYOUT]
            yield region, 384, (896, 1280), items
        else:
            items = [(i, 512 * k, 512, 0) for k, i in enumerate(chunks)]
            yield region, 512 * len(chunks), None, items


def build():
    nc = bacc.Bacc(trn_type="TRN2")
    xt = nc.dram_tensor("xt", [BPC, C, T], BF16, kind="ExternalInput")
    wqk = nc.dram_tensor("wqk", [C, 128], BF16, kind="ExternalInput")
    wv = nc.dram_tensor("wv", [C, H], BF16, kind="ExternalInput")
    mask = nc.dram_tensor("mask", [128, DIAG_COLS], BF16, kind="ExternalInput")
    ident = nc.dram_tensor("ident", [128, 128], BF16, kind="ExternalInput")
    y = nc.dram_tensor("y", [BPC, T, H], F32, kind="ExternalOutput")

    with TileContext(nc) as tc:
        with tc.tile_pool(name="const", bufs=1) as const, \
             tc.tile_pool(name="xpool", bufs=4) as xpool, \
             tc.tile_pool(name="qktp", bufs=2) as qktp, \
             tc.tile_pool(name="ktp", bufs=2) as ktp, \
             tc.tile_pool(name="vtp", bufs=3) as vtp, \
             tc.tile_pool(name="vbigp", bufs=2) as vbigp, \
             tc.tile_pool(name="pap", bufs=2) as pap, \
             tc.tile_pool(name="pbp", bufs=2) as pbp, \
             tc.tile_pool(name="osbp", bufs=3) as osbp, \
             tc.tile_pool(name="ybufp", bufs=4) as ybufp, \
             tc.tile_pool(name="recp", bufs=4) as recp, \
             tc.tile_pool(name="psA", bufs=1, space="PSUM") as psA, \
             tc.tile_pool(name="psB", bufs=1, space="PSUM") as psB, \
             tc.tile_pool(name="psQK", bufs=1, space="PSUM") as psQK, \
             tc.tile_pool(name="psV", bufs=1, space="PSUM") as psV, \
             tc.tile_pool(name="psO", bufs=2, space="PSUM") as psO:

            # ---- constants (issue off the hot engines) ----
            wqk_all = const.tile([128, NCH * 128], BF16, name="wqk_all")
            nc.sync.dma_start(
                wqk_all[:].rearrange("p (c m) -> p c m", m=128),
                wqk[:].rearrange("(c p) m -> p c m", p=128))
            wv_all = const.tile([128, NCH * H], BF16, name="wv_all")
            nc.sync.dma_start(
                wv_all[:].rearrange("p (c m) -> p c m", m=H),
                wv[:].rearrange("(c p) m -> p c m", p=128))
            mask_sb = const.tile([128, DIAG_COLS], BF16, name="mask_sb")
            nc.scalar.dma_start(mask_sb[:], mask[:])
            id_sb = const.tile([128, 128], BF16, name="id_sb")
            nc.scalar.dma_start(id_sb[:], ident[:])
            scr = const.tile([128, 8], F32, name="scr")

            # preload the exp table set while DMAs land
            nc.scalar.activation(scr[:, 0:1], id_sb[:, 0:1], EXP, scale=1.0)

            # ---- PE warmup: junk matmuls on an unwritten tile (no deps,
            # starts right after the framework preamble) to lift HAM to 8/8 ----
            junk = const.tile([128, 512], BF16, name="junk")
            nc.vector.memset(junk[:], 0.25)
            warm = psA.tile([128, 1024], F32, name="warm", tag="A")
            for w in range(34):
                nc.tensor.matmul(warm[:, 0:128], junk[:, 0:128],
                                 junk[:, 128 * (w % 4):128 * (w % 4) + 128],
                                 start=True, stop=True)

            # per-batch persistent tiles
            qkts, kts, vbigs, xgs = {}, {}, {}, {}
            for b in range(BPC):
                qkts[b] = qktp.tile([128, T], BF16, name=f"qkt{b}", tag="qkt")
                kts[b] = ktp.tile([64, T], BF16, name=f"kt{b}", tag="kt")
                vbigs[b] = vbigp.tile([128, 16 * P_O], BF16, name=f"vbig{b}",
                                      tag="vbig")
                nc.vector.memset(
                    vbigs[b][:].rearrange("p (i c) -> p i c", c=P_O)[:, :, H:P_O],
                    1.0)

            # x loads, interleaved across batches (gpsimd; pool back-pressures)
            for ts in range(NTS):
                for b in range(BPC):
                    for g in range(2):
                        xg = xpool.tile([128, 4 * 512], BF16,
                                        name=f"xg{b}_{ts}_{g}", tag=f"xg{g}")
                        src = xt[b, 512 * g:512 * (g + 1),
                                 512 * ts:512 * (ts + 1)].rearrange(
                                     "(a p) t -> p a t", p=128)
                        dst = xg[:].rearrange("p (a t) -> p a t", t=512)
                        nc.gpsimd.dma_start(dst, src)
                        xgs[(b, ts, g)] = xg

            for ts in range(NTS):
                for b in range(BPC):
                    qkt, kt, vbig = qkts[b], kts[b], vbigs[b]
                    xts = [xgs[(b, ts, cc // 4)][:, 512 * (cc % 4):512 * (cc % 4 + 1)]
                           for cc in range(NCH)]
                    # ---- QK projection ----
                    qk_ps = psQK.tile([128, 512], F32, name="qk_ps", tag="qk")
                    for c in range(NCH):
                        nc.tensor.matmul(qk_ps[:], wqk_all[:, 128 * c:128 * (c + 1)],
                                         xts[c], start=(c == 0),
                                         stop=(c == NCH - 1))
                    nc.vector.tensor_copy(qkt[:, 512 * ts:512 * (ts + 1)], qk_ps[:])
                    nc.vector.tensor_copy(kt[:, 512 * ts:512 * (ts + 1)],
                                          qkt[64:128, 512 * ts:512 * (ts + 1)])
                    # ---- V projection (column-tiled even/odd chains) ----
                    v_ps = psV.tile([128, 512], F32, name="v_ps", tag="v")
                    for c in range(NCH):
                        half = c % 2
                        nc.tensor.matmul(
                            v_ps[64 * half:64 * half + 64, :],
                            wv_all[:, H * c:H * (c + 1)], xts[c],
                            start=(c <= 1), stop=(c >= NCH - 2),
                            tile_position=(0, 64 * half),
                            skip_group_check=True)
                    vlo = vtp.tile([64, 512], F32, name="vlo", tag="vlo")
                    nc.vector.tensor_copy(vlo[:], v_ps[0:64, :])
                    vt = vtp.tile([64, 512], BF16, name="vt", tag="vt")
                    nc.vector.tensor_add(vt[:], vlo[:], v_ps[64:128, :])
                    # ---- V transpose into vbig (rides the qk psum bank) ----
                    vtr = psQK.tile([128, 4 * H], BF16, name="vtr", tag="qk")
                    for l in range(4):
                        nc.tensor.transpose(vtr[:, H * l:H * (l + 1)],
                                            vt[:, 128 * l:128 * (l + 1)],
                                            id_sb[0:64, 0:64])
                    dstv = vbig[:, P_O * 4 * ts:P_O * (4 * ts + 4)].rearrange(
                        "p (i c) -> p i c", c=P_O)[:, :, 0:H]
                    nc.vector.tensor_copy(
                        dstv, vtr[:].rearrange("p (i c) -> p i c", c=H))

                    # ---- attention for q-slice j == ts ----
                    j = ts
                    o_ps = psO.tile([P_O, 512], F32, name="o_ps", tag="o")
                    first_pv = True
                    groups = list(_expand_groups(j))
                    ngr = len(groups)
                    for gi, (region, used, mrange, items) in enumerate(groups):
                        pool, ppool, rw = ((psA, pap, 1024) if region == 'A'
                                           else (psB, pbp, 1024))
                        sreg = pool.tile([128, rw], F32, name=f"s{region}",
                                         tag=region)
                        for (i, col, w, qoff) in items:
                            nc.tensor.matmul(
                                sreg[:, col:col + w],
                                kt[:, 128 * i:128 * (i + 1)],
                                qkt[0:64, 512 * j + qoff:512 * (j + 1)],
                                start=True, stop=True)
                        pbuf = ppool.tile([128, rw], BF16, name=f"p{region}",
                                          tag=f"p{region}")
                        nc.scalar.activation(pbuf[:, 0:used], sreg[:, 0:used],
                                             EXP, scale=0.125)
                        if mrange is not None:
                            mlo, mhi = mrange
                            nc.vector.tensor_mul(pbuf[:, 0:mhi - mlo],
                                                 pbuf[:, 0:mhi - mlo],
                                                 mask_sb[:, mlo:mhi])
                        last_group = gi == ngr - 1
                        for ii, (i, col, w, qoff) in enumerate(items):
                            nc.tensor.matmul(
                                o_ps[:, qoff:512],
                                vbig[:, P_O * i:P_O * (i + 1)],
                                pbuf[:, col:col + w],
                                start=first_pv,
                                stop=(last_group and ii == len(items) - 1),
                                skip_group_check=True)
                            first_pv = False

                    # ---- finalize slice: PE transpose back, normalize ----
                    o_sb = osbp.tile([M_O, 512], BF16, name="o_sb", tag="osb")
                    nc.vector.tensor_copy(o_sb[:], o_ps[0:M_O, :])
                    f_ps = psV.tile([128, 4 * P_O], BF16, name="f_ps", tag="v")
                    for s in range(4):
                        nc.tensor.transpose(f_ps[:, P_O * s:P_O * s + M_O],
                                            o_sb[:, 128 * s:128 * (s + 1)],
                                            id_sb[0:M_O, 0:M_O])
                    rec = recp.tile([128, 4], F32, name="rec", tag="rec")
                    nc.vector.reciprocal(
                        rec[:], f_ps[:].rearrange("p (s c) -> p s c",
                                                  c=P_O)[:, :, H])
                    ysl = ybufp.tile([128, 4 * H], F32, name="ysl", tag="ysl")
                    for s in range(4):
                        nc.vector.tensor_scalar_mul(
                            ysl[:, H * s:H * (s + 1)],
                            f_ps[:, P_O * s:P_O * s + H], rec[:, s:s + 1])
                    nc.gpsimd.dma_start(
                        y[b, 512 * j:512 * (j + 1)].rearrange(
                            "(s p) h -> p s h", p=128),
                        ysl[:].rearrange("p (s h) -> p s h", h=H))

    nc.finalize()
    return nc


_NC_CACHE = None


def _get_nc():
    global _NC_CACHE
    if _NC_CACHE is None:
        _NC_CACHE = build()
    return _NC_CACHE


def _make_mask():
    # packed diag mask: chunk d at cols `col`, width w; mask[p, col+f] = f >= p
    m = np.zeros((128, DIAG_COLS), dtype=np.float32)
    p = np.arange(128)[:, None]
    for d, col, w in DIAG1_LAYOUT + [(d, c + 896, w) for d, c, w in DIAG2_LAYOUT]:
        f = np.arange(w)[None, :]
        m[:, col:col + w] = (f >= p)
    return m


def kernel(x, Wk, Wq, Wv, _trace=False, _trace_kwargs=None):
    global LAST_EXEC_TIME_NS, LAST_RESULTS
    x = np.ascontiguousarray(np.asarray(x, dtype=np.float32))
    Wk = np.asarray(Wk, dtype=np.float32)
    Wq = np.asarray(Wq, dtype=np.float32)
    Wv = np.asarray(Wv, dtype=np.float32)

    wqk = np.ascontiguousarray(
        np.concatenate([Wq.T, Wk.T], axis=1)).astype(ml_dtypes.bfloat16)
    wv = np.ascontiguousarray(Wv.T).astype(ml_dtypes.bfloat16)
    mask = _make_mask().astype(ml_dtypes.bfloat16)
    ident = np.eye(128, dtype=ml_dtypes.bfloat16)

    in_maps = []
    for core in range(NCORES):
        xb = x[BPC * core:BPC * (core + 1)]
        xtb = np.ascontiguousarray(xb.transpose(0, 2, 1)).astype(ml_dtypes.bfloat16)
        in_maps.append({"xt": xtb, "wqk": wqk, "wv": wv, "mask": mask,
                        "ident": ident})

    nc = _get_nc()
    kwargs = {}
    if _trace:
        kwargs["trace"] = True
        if _trace_kwargs:
            kwargs.update(_trace_kwargs)
    res = bass_utils.run_bass_kernel_spmd(nc, in_maps, core_ids=list(range(NCORES)),
                                          **kwargs)
    LAST_EXEC_TIME_NS = res.exec_time_ns
    LAST_RESULTS = res

    out = np.empty((B, T, H), dtype=np.float32)
    for core in range(NCORES):
        out[BPC * core:BPC * (core + 1)] = res.results[core]["y"]
    return out

